# revision 14
# baseline (speedup 1.0000x reference)
"""Bass/Trainium2 kernel for nn_BranchedPolicyNetwork.

Computes out = tanh(features @ Wr + br) where
  features: [32768, 1024] f32
  W:        [64, 2, 1024] f32  (stacked per-branch Linear(L, 2) weights)
  b:        [64, 2] f32
returning (out[..., 0], out[..., 1]) as two [32768, 64] f32 arrays.

Strategy: data-parallel over batch across 8 NeuronCores (4096 rows each).
The TensorEngine contracts over the partition dim, so features are repacked
host-side into a transposed, tile-contiguous layout (free w.r.t. HW time).

The kernel is HBM-bound: per core it must stream the 4096x1024 feature
shard in and the 128x4096 activations out.  The correctness gate is
rel_l2 < 2e-2, and a single fp16 pass measures 3.3e-4 on the real data,
so x, W and the output all travel as fp16 (one matmul term, no hi/lo
split).  Traffic/core = 8.4 MB x + 0.26 MB w + 1.05 MB out ~= 9.7 MB,
i.e. ~28 us at the measured ~341 GB/s single-ring DMA rate.
"""

import sys

for _p in ("/opt/trn_rl_repo", "/root/.axon_site"):
    if _p not in sys.path:
        sys.path.insert(0, _p)

import numpy as np

import concourse.mybir as mybir
import concourse.tile as tile
from concourse import bacc
from concourse.bass_utils import run_bass_kernel_spmd

# Problem shapes (hardcoded per contract)
B, L, A = 32768, 1024, 64
NCORES = 8
BS = B // NCORES          # 4096 batch rows per core
KO = L // 128             # 8 contraction slices
CH = 2 * A                # 128 output channels (c = k*64 + a)

F32 = mybir.dt.float32
F16 = mybir.dt.float16

# Chunk widths (batch columns per core).  1024-wide chunks minimize DMA count
# while keeping every matmul slab at N=512; with 4 chunks and bufs=4, every x
# tile has its own SBUF slot so all loads issue up front with no waits.
# The final 1024 columns are split into two 512-col chunks (each packed
# contiguously on the host, so DMA descriptors stay >=2KB/partition): the
# last chunk's epilogue (final sub-DMA wait + act + store + completion
# receipt) is then a short 512-col tail, while the second-to-last chunk's
# epilogue still hides under the final chunk's stream.  512 (not smaller)
# because the end of the run is power-throttled and HAM-cold: a cold
# 512-col matmul consumes x at ~213 GB/s, matching the throttled stream,
# whereas 256-col tails make the cold PE the serial bottleneck.
CHUNKS = [1024, 1024, 1024, 512, 512]
assert sum(CHUNKS) == BS
CN_MAX = max(CHUNKS)
MM_N = 512  # moving free dim per matmul (fp16 cap / one fp32 PSUM bank)


_NC = None


def _build_nc():
    nc = bacc.Bacc()
    # x is packed chunk-major on the host: for each chunk (cn columns), the
    # per-partition bytes are one contiguous (ko, n) block of KO*cn elements.
    xh = nc.dram_tensor("xh", [128, KO * BS], F16, kind="ExternalInput")
    wh = nc.dram_tensor("wh", [128, KO, CH], F16, kind="ExternalInput")
    bvec = nc.dram_tensor("bias", [CH, 1], F32, kind="ExternalInput")
    out = nc.dram_tensor("out", [CH, BS], F16, kind="ExternalOutput")

    with tile.TileContext(nc) as tc:
        with (
            tc.tile_pool(name="consts", bufs=1) as consts,
            tc.tile_pool(name="xhp", bufs=5) as xhp,
            tc.tile_pool(name="op", bufs=3) as op,
            tc.tile_pool(name="ps", bufs=3, space="PSUM") as ps,
            tc.tile_pool(name="warm", bufs=1, space="PSUM") as warm_ps,
        ):
            # PE warmup: ~10 dependency-free matmuls on zeroed tiles fill the
            # otherwise-idle window while the first loads stream in, so the
            # HAM clock gate is already at 8/8 (2.4 GHz) when real matmuls
            # start (saves the ~2x-slow cold ramp on the critical path).
            w_warm = consts.tile([128, CH], F16)
            nc.vector.memset(w_warm[:], 0.0)
            x_warm = consts.tile([128, MM_N], F16)
            nc.gpsimd.memset(x_warm[:], 0.0)
            pw = warm_ps.tile([CH, MM_N], F32)
            for i in range(10):
                nc.tensor.matmul(
                    pw[:], w_warm[:], x_warm[:], start=(i == 0), stop=(i == 9)
                )
            # Ring assignment: the Sync (SP) HWDGE ring is purely the x
            # stream in need-order.  The Scalar (ACT) ring loads the small
            # constants up front (before any ACTIVATE exists, so no convoy),
            # then does activations + out-stores; a store depends on its own
            # activation, so no convoy can form there either.
            wh_sb = consts.tile([128, KO, CH], F16)
            nc.scalar.dma_start(wh_sb[:], wh[:])
            b_sb = consts.tile([CH, 1], F32)
            nc.scalar.dma_start(b_sb[:], bvec[:])

            # Issue ALL x loads up front on the Sync ring: with bufs matching
            # the chunk count, every x tile has its own SBUF slot, so no load
            # ever waits on a tile release and the ring streams continuously
            # at HBM rate.  (Measured: one HWDGE ring saturates HBM by
            # itself; splitting the stream across rings was slower, and
            # descriptors below 2KB/partition collapse the DMA rate.)
            xh_tiles = []
            n0 = 0
            for ci, cn in enumerate(CHUNKS):
                off = KO * n0
                src_h = xh[:, off : off + KO * cn].rearrange(
                    "p (ko n) -> p ko n", ko=KO
                )
                # Exact-width tile: a CN_MAX-padded tile would break the
                # SBUF-side contiguity for narrow chunks and shrink DMA
                # descriptors below the ~2KB line-rate knee (measured: 1KB
                # descriptors collapse the stream to ~100-270 GB/s).
                xh_sb = xhp.tile([128, KO, cn], F16, tag="xh", name="xh_sb")
                # ko-major sub-DMAs (256-512 KB, 2-4KB/partition
                # descriptors): Tile tracks deps per region, so the chunk's
                # first matmuls start as soon as the first ko slices land.
                hs = 2
                for k0 in range(0, KO, hs):
                    nc.sync.dma_start(
                        xh_sb[:, k0 : k0 + hs], src_h[:, k0 : k0 + hs]
                    )
                xh_tiles.append(xh_sb)
                n0 += cn

            n0 = 0
            for ci, cn in enumerate(CHUNKS):
                xh_sb = xh_tiles[ci]
                pt = ps.tile([CH, cn], F32, tag="pt", name="pt")
                o_sb = op.tile([CH, cn], F16, tag="o", name="o_sb")
                for s0 in range(0, cn, MM_N):
                    s1 = min(s0 + MM_N, cn)
                    for ko in range(KO):
                        # start/stop are per PSUM slab (bank region)
                        nc.tensor.matmul(
                            pt[:, s0:s1],
                            wh_sb[:, ko],
                            xh_sb[:, ko, s0:s1],
                            start=(ko == 0),
                            stop=(ko == KO - 1),
                        )
                nc.scalar.activation(
                    o_sb[:],
                    pt[:],
                    mybir.ActivationFunctionType.Tanh,
                    bias=b_sb[:, 0:1],
                    scale=1.0,
                )
                # Stores ride the ACT engine's HWDGE ring (the store depends
                # on its activation anyway, and the Sync ring is busy
                # streaming x) — EXCEPT the final chunk's store, which goes
                # on the by-then-idle Sync ring so its descriptors don't
                # queue behind the previous chunk's store at the very end.
                if ci == len(CHUNKS) - 1:
                    nc.sync.dma_start(out[:, n0 : n0 + cn], o_sb[:])
                else:
                    nc.scalar.dma_start(out[:, n0 : n0 + cn], o_sb[:])
                n0 += cn
    nc.compile()
    return nc


def _get_nc():
    global _NC
    if _NC is None:
        _NC = _build_nc()
    return _NC


def _pack_x(shard16):
    # shard16 [BS, L] -> chunk-major [128, KO*BS]: per partition p, chunk c
    # occupies a contiguous (ko, n) block.
    shT = shard16.T  # [L, BS] view
    parts = []
    n0 = 0
    for cn in CHUNKS:
        blk = (
            shT[:, n0 : n0 + cn]
            .reshape(KO, 128, cn)
            .transpose(1, 0, 2)
            .reshape(128, KO * cn)
        )
        parts.append(blk)
        n0 += cn
    return np.ascontiguousarray(np.concatenate(parts, axis=1))


def _shard_inputs(features, W, b):
    features = np.ascontiguousarray(features, dtype=np.float32)
    W = np.ascontiguousarray(W, dtype=np.float32)
    b = np.ascontiguousarray(b, dtype=np.float32)

    # Wr[l, c] with c = k*A + a; fp16, device layout [p, ko, c]
    wr = W.transpose(2, 1, 0).reshape(L, CH)
    wr_h = wr.astype(np.float16)
    wh_dev = np.ascontiguousarray(wr_h.reshape(KO, 128, CH).transpose(1, 0, 2))
    b_dev = np.ascontiguousarray(b.transpose(1, 0).reshape(CH, 1))

    in_maps = []
    for i in range(NCORES):
        sh = features[i * BS : (i + 1) * BS]  # [BS, L]
        sh_h = sh.astype(np.float16)
        in_maps.append(
            {
                "xh": _pack_x(sh_h),
                "wh": wh_dev,
                "bias": b_dev,
            }
        )
    return in_maps


def _gather(results):
    out0 = np.empty((B, A), dtype=np.float32)
    out1 = np.empty((B, A), dtype=np.float32)
    for i, r in enumerate(results):
        arr = r["out"].T.astype(np.float32)  # [CH, BS] f16 -> [BS, CH] f32
        out0[i * BS : (i + 1) * BS] = arr[:, :A]
        out1[i * BS : (i + 1) * BS] = arr[:, A:]
    return out0, out1


def _run(inputs, trace=False, trace_cores=None):
    nc = _get_nc()
    in_maps = _shard_inputs(inputs["features"], inputs["W"], inputs["b"])
    res = run_bass_kernel_spmd(
        nc,
        in_maps,
        core_ids=list(range(NCORES)),
        trace=trace,
        trace_cores=trace_cores,
    )
    return _gather(res.results), res


def kernel(features, W, b):
    (out0, out1), _ = _run({"features": features, "W": W, "b": b})
    return out0, out1


# revision 15
# speedup vs baseline: 1.0116x; 1.0116x over previous
"""Bass/Trainium2 kernel for nn_BranchedPolicyNetwork.

Computes out = tanh(features @ Wr + br) where
  features: [32768, 1024] f32
  W:        [64, 2, 1024] f32  (stacked per-branch Linear(L, 2) weights)
  b:        [64, 2] f32
returning (out[..., 0], out[..., 1]) as two [32768, 64] f32 arrays.

Strategy: data-parallel over batch across 8 NeuronCores (4096 rows each).
The TensorEngine contracts over the partition dim, so features are repacked
host-side into a transposed, tile-contiguous layout (free w.r.t. HW time).

The kernel is HBM-bound: per core it must stream the 4096x1024 feature
shard in and the 128x4096 activations out.  The correctness gate is
rel_l2 < 2e-2, and a single fp16 pass measures 3.3e-4 on the real data,
so x, W and the output all travel as fp16 (one matmul term, no hi/lo
split).  Traffic/core = 8.4 MB x + 0.26 MB w + 1.05 MB out ~= 9.7 MB,
i.e. ~28 us at the measured ~341 GB/s single-ring DMA rate.
"""

import sys

for _p in ("/opt/trn_rl_repo", "/root/.axon_site"):
    if _p not in sys.path:
        sys.path.insert(0, _p)

import numpy as np

import concourse.mybir as mybir
import concourse.tile as tile
from concourse import bacc
from concourse.bass_utils import run_bass_kernel_spmd

# Problem shapes (hardcoded per contract)
B, L, A = 32768, 1024, 64
NCORES = 8
BS = B // NCORES          # 4096 batch rows per core
KO = L // 128             # 8 contraction slices
CH = 2 * A                # 128 output channels (c = k*64 + a)

F32 = mybir.dt.float32
F16 = mybir.dt.float16

# Chunk widths (batch columns per core).  1024-wide chunks minimize DMA count
# while keeping every matmul slab at N=512; with bufs matching the chunk
# count, every x tile has its own SBUF slot so all loads issue up front.
# The final 1024 columns are split into two 512-col chunks (each packed
# contiguously on the host, so DMA descriptors stay >=2KB/partition): the
# last chunk's epilogue (final sub-DMA wait + act + store + completion
# receipt) is then a short 512-col tail, while the second-to-last chunk's
# epilogue still hides under the final chunk's stream.  512 (not smaller)
# because the end of the run is power-throttled and HAM-cold: a cold
# 512-col matmul consumes x at ~213 GB/s, matching the throttled stream,
# whereas 256-col tails make the cold PE the serial bottleneck.
CHUNKS = [1024, 1024, 1024, 512, 512]
assert sum(CHUNKS) == BS
CN_MAX = max(CHUNKS)
MM_N = 512  # moving free dim per matmul (fp16 cap / one fp32 PSUM bank)


_NC = None


def _build_nc():
    nc = bacc.Bacc()
    # x is packed chunk-major on the host: for each chunk (cn columns), the
    # per-partition bytes are one contiguous (ko, n) block of KO*cn elements.
    xh = nc.dram_tensor("xh", [128, KO * BS], F16, kind="ExternalInput")
    wh = nc.dram_tensor("wh", [128, KO, CH], F16, kind="ExternalInput")
    bvec = nc.dram_tensor("bias", [CH, 1], F32, kind="ExternalInput")
    out = nc.dram_tensor("out", [CH, BS], F16, kind="ExternalOutput")

    with tile.TileContext(nc) as tc:
        with (
            tc.tile_pool(name="consts", bufs=1) as consts,
            tc.tile_pool(name="xhp", bufs=5) as xhp,
            tc.tile_pool(name="op", bufs=3) as op,
            tc.tile_pool(name="ps", bufs=3, space="PSUM") as ps,
            tc.tile_pool(name="warm", bufs=1, space="PSUM") as warm_ps,
        ):
            # PE warmup: ~10 dependency-free matmuls on zeroed tiles fill the
            # otherwise-idle window while the first loads stream in, so the
            # HAM clock gate is already at 8/8 (2.4 GHz) when real matmuls
            # start (saves the ~2x-slow cold ramp on the critical path).
            w_warm = consts.tile([128, CH], F16)
            nc.vector.memset(w_warm[:], 0.0)
            x_warm = consts.tile([128, MM_N], F16)
            nc.gpsimd.memset(x_warm[:], 0.0)
            pw = warm_ps.tile([CH, MM_N], F32)
            for i in range(10):
                nc.tensor.matmul(
                    pw[:], w_warm[:], x_warm[:], start=(i == 0), stop=(i == 9)
                )
            # Ring assignment: the Sync (SP) HWDGE ring is purely the x
            # stream in need-order.  The Scalar (ACT) ring loads the small
            # constants up front (before any ACTIVATE exists, so no convoy),
            # then does activations + out-stores; a store depends on its own
            # activation, so no convoy can form there either.
            wh_sb = consts.tile([128, KO, CH], F16)
            nc.scalar.dma_start(wh_sb[:], wh[:])
            b_sb = consts.tile([CH, 1], F32)
            nc.scalar.dma_start(b_sb[:], bvec[:])

            # Issue ALL x loads up front on the Sync ring: with bufs matching
            # the chunk count, every x tile has its own SBUF slot, so no load
            # ever waits on a tile release and the ring streams continuously
            # at HBM rate.  (Measured: one HWDGE ring saturates HBM by
            # itself; splitting the stream across rings was slower, and
            # descriptors below 2KB/partition collapse the DMA rate.)
            xh_tiles = []
            n0 = 0
            for ci, cn in enumerate(CHUNKS):
                off = KO * n0
                src_h = xh[:, off : off + KO * cn].rearrange(
                    "p (ko n) -> p ko n", ko=KO
                )
                # Exact-width tile: a CN_MAX-padded tile would break the
                # SBUF-side contiguity for narrow chunks and shrink DMA
                # descriptors below the ~2KB line-rate knee (measured: 1KB
                # descriptors collapse the stream to ~100-270 GB/s).
                xh_sb = xhp.tile([128, KO, cn], F16, tag="xh", name="xh_sb")
                # ko-major sub-DMAs (256-512 KB, 2-4KB/partition
                # descriptors): Tile tracks deps per region, so the chunk's
                # first matmuls start as soon as the first ko slices land.
                hs = 2
                for k0 in range(0, KO, hs):
                    nc.sync.dma_start(
                        xh_sb[:, k0 : k0 + hs], src_h[:, k0 : k0 + hs]
                    )
                xh_tiles.append(xh_sb)
                n0 += cn

            n0 = 0
            for ci, cn in enumerate(CHUNKS):
                xh_sb = xh_tiles[ci]
                pt = ps.tile([CH, cn], F32, tag="pt", name="pt")
                o_sb = op.tile([CH, cn], F16, tag="o", name="o_sb")
                for s0 in range(0, cn, MM_N):
                    s1 = min(s0 + MM_N, cn)
                    for ko in range(KO):
                        # start/stop are per PSUM slab (bank region)
                        nc.tensor.matmul(
                            pt[:, s0:s1],
                            wh_sb[:, ko],
                            xh_sb[:, ko, s0:s1],
                            start=(ko == 0),
                            stop=(ko == KO - 1),
                        )
                nc.scalar.activation(
                    o_sb[:],
                    pt[:],
                    mybir.ActivationFunctionType.Tanh,
                    bias=b_sb[:, 0:1],
                    scale=1.0,
                )
                # Stores ride the ACT engine's HWDGE ring (the store depends
                # on its activation anyway, and the Sync ring is busy
                # streaming x) — EXCEPT the final chunk's store, which goes
                # on the by-then-idle Sync ring so its descriptors don't
                # queue behind the previous chunk's store at the very end.
                if ci == len(CHUNKS) - 1:
                    nc.sync.dma_start(out[:, n0 : n0 + cn], o_sb[:])
                else:
                    nc.scalar.dma_start(out[:, n0 : n0 + cn], o_sb[:])
                n0 += cn
    nc.compile()
    return nc


def _get_nc():
    global _NC
    if _NC is None:
        _NC = _build_nc()
    return _NC


def _pack_x(shard16):
    # shard16 [BS, L] -> chunk-major [128, KO*BS]: per partition p, chunk c
    # occupies a contiguous (ko, n) block.
    shT = shard16.T  # [L, BS] view
    parts = []
    n0 = 0
    for cn in CHUNKS:
        blk = (
            shT[:, n0 : n0 + cn]
            .reshape(KO, 128, cn)
            .transpose(1, 0, 2)
            .reshape(128, KO * cn)
        )
        parts.append(blk)
        n0 += cn
    return np.ascontiguousarray(np.concatenate(parts, axis=1))


def _shard_inputs(features, W, b):
    features = np.ascontiguousarray(features, dtype=np.float32)
    W = np.ascontiguousarray(W, dtype=np.float32)
    b = np.ascontiguousarray(b, dtype=np.float32)

    # Wr[l, c] with c = k*A + a; fp16, device layout [p, ko, c]
    wr = W.transpose(2, 1, 0).reshape(L, CH)
    wr_h = wr.astype(np.float16)
    wh_dev = np.ascontiguousarray(wr_h.reshape(KO, 128, CH).transpose(1, 0, 2))
    b_dev = np.ascontiguousarray(b.transpose(1, 0).reshape(CH, 1))

    in_maps = []
    for i in range(NCORES):
        sh = features[i * BS : (i + 1) * BS]  # [BS, L]
        sh_h = sh.astype(np.float16)
        in_maps.append(
            {
                "xh": _pack_x(sh_h),
                "wh": wh_dev,
                "bias": b_dev,
            }
        )
    return in_maps


def _gather(results):
    out0 = np.empty((B, A), dtype=np.float32)
    out1 = np.empty((B, A), dtype=np.float32)
    for i, r in enumerate(results):
        arr = r["out"].T.astype(np.float32)  # [CH, BS] f16 -> [BS, CH] f32
        out0[i * BS : (i + 1) * BS] = arr[:, :A]
        out1[i * BS : (i + 1) * BS] = arr[:, A:]
    return out0, out1


def _run(inputs, trace=False, trace_cores=None):
    nc = _get_nc()
    in_maps = _shard_inputs(inputs["features"], inputs["W"], inputs["b"])
    res = run_bass_kernel_spmd(
        nc,
        in_maps,
        core_ids=list(range(NCORES)),
        trace=trace,
        trace_cores=trace_cores,
    )
    return _gather(res.results), res


def kernel(features, W, b):
    (out0, out1), _ = _run({"features": features, "W": W, "b": b})
    return out0, out1
